# revision 4
# baseline (speedup 1.0000x reference)
"""Autoregressive flow sampler — data-parallel across 8 NeuronCores.

Sharding strategy (per sharding_hint): pure data parallel. The batch dim
(B=1024) of `gumbel` plus the per-sample KV caches and alpha/beta counters
are sharded 128-per-core across the 8 cores; the tiny d_model=128
transformer weights are replicated on every core.

One decode *step* (4 transformer layers + gumbel-max sampling) is compiled
as a single pmapped program; the 64-step autoregressive loop is driven from
Python with the carried state (tok, counters, KV caches) resident and
sharded on the devices. Compiling per-step instead of the whole scan keeps
the neuronx-cc compile tractable.
"""

import numpy as np
import jax
import jax.numpy as jnp

# Fixed by module source / config defaults (hardcoded per contract).
L, H, D, DFF, T, B = 4, 4, 128, 512, 64, 1024
S, V, BOS = 4, 5, 4
DH = D // H
M = 8  # cores


def _ln(x, w, b):
    mu = x.mean(-1, keepdims=True)
    var = ((x - mu) ** 2).mean(-1, keepdims=True)
    return (x - mu) / jnp.sqrt(var + 1e-5) * w + b


def _sample(logits, g, a_rem, b_rem, i):
    """Validity mask + gumbel-max sampling, argmax via single-operand
    reduces (neuronxcc rejects the variadic reduce jnp.argmax lowers to)."""
    oa = T - i - 1
    m = jnp.stack([
        (a_rem <= oa) & (b_rem <= oa),
        (b_rem > 0) & (a_rem <= oa) & (b_rem - 1 <= oa),
        (a_rem > 0) & (a_rem - 1 <= oa) & (b_rem <= oa),
        (a_rem > 0) & (b_rem > 0) & (a_rem - 1 <= oa) & (b_rem - 1 <= oa)], -1)
    logp = jax.nn.log_softmax(jnp.where(m, logits, -jnp.inf), -1)
    y = logp + g
    mx = jnp.max(y, -1, keepdims=True)
    iota = jnp.arange(S, dtype=jnp.int32)[None, :]
    s = jnp.min(jnp.where(y >= mx, iota, S), -1).astype(jnp.int32)
    lp = jnp.sum(jnp.where(iota == s[:, None], logp, 0.0), -1)
    return s, lp


def _step(tok, a_rem, b_rem, Kc, Vc, g, i, state_emb, pos_emb, ln1_w, ln1_b,
          wqkv, bqkv, wo, bo, ln2_w, ln2_b, w1, b1, w2, b2, fn_w, fn_b,
          head_w, head_b):
    Bl = tok.shape[0]
    x = state_emb[tok] + pos_emb[i]
    for l in range(L):
        xn = _ln(x, ln1_w[l], ln1_b[l])
        q, k, v = jnp.split(xn @ wqkv[l].T + bqkv[l], 3, axis=-1)
        Kc = jax.lax.dynamic_update_slice(Kc, k[:, None, :][None], (l, 0, i, 0))
        Vc = jax.lax.dynamic_update_slice(Vc, v[:, None, :][None], (l, 0, i, 0))
        qh = q.reshape(Bl, H, DH)
        kh = Kc[l].reshape(Bl, T, H, DH)
        vh = Vc[l].reshape(Bl, T, H, DH)
        sc = jnp.einsum('bhd,bthd->bht', qh, kh) / jnp.sqrt(jnp.float32(DH))
        sc = jnp.where((jnp.arange(T) <= i)[None, None, :], sc, -jnp.inf)
        ctx = jnp.einsum('bht,bthd->bhd', jax.nn.softmax(sc, -1), vh).reshape(Bl, D)
        x = x + ctx @ wo[l].T + bo[l]
        h = jax.nn.gelu(_ln(x, ln2_w[l], ln2_b[l]) @ w1[l].T + b1[l],
                        approximate=False)
        x = x + h @ w2[l].T + b2[l]
    logits = _ln(x, fn_w, fn_b) @ head_w.T + head_b
    s, lp = _sample(logits, g, a_rem, b_rem, i)
    return s, a_rem - (s >> 1), b_rem - (s & 1), Kc, Vc, lp


_IN_AXES = (0, 0, 0, 0, 0, 0) + (None,) * 19
_pstep = jax.pmap(_step, in_axes=_IN_AXES)


def _decode_local(gumbel, state_emb, pos_emb, ln1_w, ln1_b, wqkv, bqkv, wo, bo,
                  ln2_w, ln2_b, w1, b1, w2, b2, fn_w, fn_b, head_w, head_b,
                  n_alpha, n_beta):
    """Whole-scan single-shard decode; used by the CPU fallback path."""
    Bl = gumbel.shape[0]
    w = (state_emb, pos_emb, ln1_w, ln1_b, wqkv, bqkv, wo, bo,
         ln2_w, ln2_b, w1, b1, w2, b2, fn_w, fn_b, head_w, head_b)

    def step(carry, xs):
        tok, a_rem, b_rem, Kc, Vc = carry
        g, i = xs
        s, a2, b2_, Kc, Vc, lp = _step(tok, a_rem, b_rem, Kc, Vc, g, i, *w)
        return (s, a2, b2_, Kc, Vc), (s, lp)

    tok0 = jnp.full((Bl,), BOS, jnp.int32)
    a0 = jnp.full((Bl,), n_alpha, jnp.int32)
    b0 = jnp.full((Bl,), n_beta, jnp.int32)
    Kc0 = jnp.zeros((L, Bl, T, D), jnp.float32)
    Vc0 = jnp.zeros((L, Bl, T, D), jnp.float32)
    _, (states, lps) = jax.lax.scan(
        step, (tok0, a0, b0, Kc0, Vc0),
        (gumbel.transpose(1, 0, 2), jnp.arange(T)))
    return states.T, lps.sum(0)


def kernel(gumbel, state_emb, pos_emb, ln1_w, ln1_b, wqkv, bqkv, wo, bo,
           ln2_w, ln2_b, w1, b1, w2, b2, fn_w, fn_b, head_w, head_b,
           n_alpha, n_beta):
    Bl = B // M
    g = np.ascontiguousarray(np.asarray(gumbel, np.float32)).reshape(M, Bl, T, S)
    na = np.int32(np.asarray(n_alpha))
    nb = np.int32(np.asarray(n_beta))
    w = tuple(np.asarray(a, np.float32) for a in (
        state_emb, pos_emb, ln1_w, ln1_b, wqkv, bqkv, wo, bo,
        ln2_w, ln2_b, w1, b1, w2, b2, fn_w, fn_b, head_w, head_b))
    try:
        tok = np.full((M, Bl), BOS, np.int32)
        a_rem = np.full((M, Bl), na, np.int32)
        b_rem = np.full((M, Bl), nb, np.int32)
        Kc = np.zeros((M, L, Bl, T, D), np.float32)
        Vc = np.zeros((M, L, Bl, T, D), np.float32)
        states, lp_tot = [], 0.0
        for i in range(T):
            tok, a_rem, b_rem, Kc, Vc, lp = _pstep(
                tok, a_rem, b_rem, Kc, Vc, g[:, :, i, :],
                np.int32(i), *w)
            states.append(tok)
            lp_tot = lp_tot + lp
        states = np.stack([np.asarray(s) for s in states], -1)  # (M, Bl, T)
        lp = np.asarray(lp_tot)
    except Exception:
        # Fallback: same sharded computation chunk-by-chunk on CPU.
        cpu = jax.devices('cpu')[0]
        fn = jax.jit(_decode_local, device=cpu)
        outs = [fn(g[i], *w, na, nb) for i in range(M)]
        states = np.stack([np.asarray(o[0]) for o in outs])
        lp = np.stack([np.asarray(o[1]) for o in outs])
    return states.reshape(B, T).astype(np.int32), lp.reshape(B).astype(np.float32)


# revision 8
# speedup vs baseline: 2.5014x; 2.5014x over previous
"""Autoregressive flow sampler — data-parallel across 8 NeuronCores.

Sharding strategy (per sharding_hint): pure data parallel. The batch dim
(B=1024) of `gumbel` plus the per-sample KV caches and alpha/beta counters
are sharded 128-per-core across the 8 cores; the tiny d_model=128
transformer weights are replicated on every core.

One decode *step* (4 transformer layers + gumbel-max sampling) is compiled
as a single pmapped program; the 64-step autoregressive loop is driven from
Python with the carried state (tok, counters, KV caches) resident and
sharded on the devices. Compiling per-step instead of the whole scan keeps
the neuronx-cc compile tractable.
"""

import numpy as np
import jax
import jax.numpy as jnp

# Fixed by module source / config defaults (hardcoded per contract).
L, H, D, DFF, T, B = 4, 4, 128, 512, 64, 1024
S, V, BOS = 4, 5, 4
DH = D // H
M = 8  # cores


def _ln(x, w, b):
    mu = x.mean(-1, keepdims=True)
    var = ((x - mu) ** 2).mean(-1, keepdims=True)
    return (x - mu) / jnp.sqrt(var + 1e-5) * w + b


def _sample(logits, g, a_rem, b_rem, i):
    """Validity mask + gumbel-max sampling, argmax via single-operand
    reduces (neuronxcc rejects the variadic reduce jnp.argmax lowers to)."""
    oa = T - i - 1
    m = jnp.stack([
        (a_rem <= oa) & (b_rem <= oa),
        (b_rem > 0) & (a_rem <= oa) & (b_rem - 1 <= oa),
        (a_rem > 0) & (a_rem - 1 <= oa) & (b_rem <= oa),
        (a_rem > 0) & (b_rem > 0) & (a_rem - 1 <= oa) & (b_rem - 1 <= oa)], -1)
    logp = jax.nn.log_softmax(jnp.where(m, logits, -jnp.inf), -1)
    y = logp + g
    mx = jnp.max(y, -1, keepdims=True)
    iota = jnp.arange(S, dtype=jnp.int32)[None, :]
    s = jnp.min(jnp.where(y >= mx, iota, S), -1).astype(jnp.int32)
    lp = jnp.sum(jnp.where(iota == s[:, None], logp, 0.0), -1)
    return s, lp


def _step(tok, a_rem, b_rem, Kc, Vc, lp_acc, g_all, i, state_emb, pos_emb,
          ln1_w, ln1_b, wqkv, bqkv, wo, bo, ln2_w, ln2_b, w1, b1, w2, b2,
          fn_w, fn_b, head_w, head_b):
    Bl = tok.shape[0]
    g = jax.lax.dynamic_slice(g_all, (0, i, 0), (Bl, 1, S))[:, 0, :]
    x = state_emb[tok] + pos_emb[i]
    for l in range(L):
        xn = _ln(x, ln1_w[l], ln1_b[l])
        q, k, v = jnp.split(xn @ wqkv[l].T + bqkv[l], 3, axis=-1)
        Kc = jax.lax.dynamic_update_slice(Kc, k[:, None, :][None], (l, 0, i, 0))
        Vc = jax.lax.dynamic_update_slice(Vc, v[:, None, :][None], (l, 0, i, 0))
        qh = q.reshape(Bl, H, DH)
        kh = Kc[l].reshape(Bl, T, H, DH)
        vh = Vc[l].reshape(Bl, T, H, DH)
        sc = jnp.einsum('bhd,bthd->bht', qh, kh) / jnp.sqrt(jnp.float32(DH))
        sc = jnp.where((jnp.arange(T) <= i)[None, None, :], sc, -jnp.inf)
        ctx = jnp.einsum('bht,bthd->bhd', jax.nn.softmax(sc, -1), vh).reshape(Bl, D)
        x = x + ctx @ wo[l].T + bo[l]
        h = jax.nn.gelu(_ln(x, ln2_w[l], ln2_b[l]) @ w1[l].T + b1[l],
                        approximate=False)
        x = x + h @ w2[l].T + b2[l]
    logits = _ln(x, fn_w, fn_b) @ head_w.T + head_b
    s, lp = _sample(logits, g, a_rem, b_rem, i)
    return s, a_rem - (s >> 1), b_rem - (s & 1), Kc, Vc, lp_acc + lp


_IN_AXES = (0,) * 7 + (None,) + (0,) * 18
_pstep = jax.pmap(_step, in_axes=_IN_AXES)
_pzeros = jax.pmap(lambda _: (jnp.zeros((L, B // M, T, D), jnp.float32),
                              jnp.zeros((L, B // M, T, D), jnp.float32),
                              jnp.zeros((B // M,), jnp.float32)))


def _decode_local(gumbel, state_emb, pos_emb, ln1_w, ln1_b, wqkv, bqkv, wo, bo,
                  ln2_w, ln2_b, w1, b1, w2, b2, fn_w, fn_b, head_w, head_b,
                  n_alpha, n_beta):
    """Whole-scan single-shard decode; used by the CPU fallback path."""
    Bl = gumbel.shape[0]
    w = (state_emb, pos_emb, ln1_w, ln1_b, wqkv, bqkv, wo, bo,
         ln2_w, ln2_b, w1, b1, w2, b2, fn_w, fn_b, head_w, head_b)

    zl = jnp.zeros((Bl,), jnp.float32)

    def step(carry, i):
        tok, a_rem, b_rem, Kc, Vc = carry
        s, a2, b2_, Kc, Vc, lp = _step(
            tok, a_rem, b_rem, Kc, Vc, zl, gumbel, i, *w)
        return (s, a2, b2_, Kc, Vc), (s, lp)

    tok0 = jnp.full((Bl,), BOS, jnp.int32)
    a0 = jnp.full((Bl,), n_alpha, jnp.int32)
    b0 = jnp.full((Bl,), n_beta, jnp.int32)
    Kc0 = jnp.zeros((L, Bl, T, D), jnp.float32)
    Vc0 = jnp.zeros((L, Bl, T, D), jnp.float32)
    _, (states, lps) = jax.lax.scan(
        step, (tok0, a0, b0, Kc0, Vc0), jnp.arange(T))
    return states.T, lps.sum(0)


def kernel(gumbel, state_emb, pos_emb, ln1_w, ln1_b, wqkv, bqkv, wo, bo,
           ln2_w, ln2_b, w1, b1, w2, b2, fn_w, fn_b, head_w, head_b,
           n_alpha, n_beta):
    Bl = B // M
    g = np.ascontiguousarray(np.asarray(gumbel, np.float32)).reshape(M, Bl, T, S)
    na = np.int32(np.asarray(n_alpha))
    nb = np.int32(np.asarray(n_beta))
    w = tuple(np.asarray(a, np.float32) for a in (
        state_emb, pos_emb, ln1_w, ln1_b, wqkv, bqkv, wo, bo,
        ln2_w, ln2_b, w1, b1, w2, b2, fn_w, fn_b, head_w, head_b))
    try:
        devs = jax.local_devices()[:M]
        # One-time host->device transfers: gumbel sharded, weights replicated.
        g_dev = jax.device_put_sharded([g[i] for i in range(M)], devs)
        w_dev = [jax.device_put_replicated(a, devs) for a in w]
        Kc, Vc, lp = _pzeros(np.zeros((M, 1), np.float32))
        tok = np.full((M, Bl), BOS, np.int32)
        a_rem = np.full((M, Bl), na, np.int32)
        b_rem = np.full((M, Bl), nb, np.int32)
        states = []
        for i in range(T):
            tok, a_rem, b_rem, Kc, Vc, lp = _pstep(
                tok, a_rem, b_rem, Kc, Vc, lp, g_dev, np.int32(i), *w_dev)
            states.append(tok)
        states = np.stack([np.asarray(s) for s in states], -1)  # (M, Bl, T)
        lp = np.asarray(lp)
    except Exception:
        # Fallback: same sharded computation chunk-by-chunk on CPU.
        cpu = jax.devices('cpu')[0]
        fn = jax.jit(_decode_local, device=cpu)
        outs = [fn(g[i], *w, na, nb) for i in range(M)]
        states = np.stack([np.asarray(o[0]) for o in outs])
        lp = np.stack([np.asarray(o[1]) for o in outs])
    return states.reshape(B, T).astype(np.int32), lp.reshape(B).astype(np.float32)
